# revision 19
# baseline (speedup 1.0000x reference)
"""Self-contained Trainium2 kernel for nn_AttentionLayer_53515292508185.

kernel(**inputs) takes the FULL inputs (B=4, N=2048, D=256, H=4, E=256),
shards across 8 NeuronCores, runs one SPMD Bass graph, and gathers the
full [B, N, E] output.

Sharding: core c -> (batch b = c//2, query-row half r = c%2). Each core
computes all 4 heads for 1024 query rows of one batch, recomputing K/V
over the full 2048 keys locally -> zero collectives. The host np.rolls
x/mask per core so that core's query rows sit at rows [0, 1024).

fp8 redesign (vs the f32r baseline):
  - All attention matmuls run in fp8e4m3 with MatmulPerfMode.DoubleRow:
    K=256 contraction in ONE instruction at 0.5 cyc/row (4x f32r).
    Projections q/k/v/gate, scores, att@v all DoubleRow; out-proj in bf16.
  - One activation table the whole kernel (exp_and_others): softmax exp
    AND the sigmoid gate via sigmoid(z) = 0.5*(1+tanh(z/2)), LN rsqrt via
    DVE pow(v, -0.5). Zero act-table reloads.
  - P^T (exp of scores) written straight from wide [128,1024] PSUM to fp8
    SBUF by the Act engine, with the key-mask as a per-partition bias.
  - The softmax denominator comes free from two ones-columns in v.
  - Epilogue fused: u=(1+tanh)*po in one scalar_tensor_tensor,
    a4=(u*rmask)+x_bf16 in another; bn_stats/bn_aggr for LN stats; LN
    applies distributed across Act/DVE/Pool (Pool = gpsimd queue cannot
    read PSUM, so it only gets SBUF->SBUF work).
  - Residual of the out-projection added INSIDE PSUM via an identity
    matmul of x_bf16 (frees a DVE pass).

LN affine params (g_*/b_*) and out_b are ones/zeros per the spec and are
folded away. They are accepted and ignored.
"""

import numpy as np

B, N, D, H, E = 4, 2048, 256, 4, 256
NR = N // 2           # query rows per core
NCORES = 8
EPS = 1e-6

_cache = {}

# Tunables for A/B experiments. pool_level: 2=heavy Pool use, 1=Pool for
# weight converts/xbf only, 0=Pool for DMA only.
CFG = {"pool_level": 1, "conv_act_every": 4}


def _build(n_reps=1):
    pool_level = CFG["pool_level"]
    conv_act_every = CFG["conv_act_every"]
    from contextlib import ExitStack
    import concourse.mybir as mybir
    import concourse.tile as tile
    from concourse import bacc
    from concourse.masks import make_identity

    F32 = mybir.dt.float32
    BF16 = mybir.dt.bfloat16
    FP8 = mybir.dt.float8e4
    U8 = mybir.dt.uint8
    AF = mybir.ActivationFunctionType
    ALU = mybir.AluOpType
    DR = mybir.MatmulPerfMode.DoubleRow

    Nr = NR
    MT = N // 128           # key tiles (16)
    NRT = Nr // 128         # query-row groups per core (8)
    scale = 1.0 / float(E) ** 0.5

    nc = bacc.Bacc(None, target_bir_lowering=False, debug=False)

    x_ext = nc.declare_dram_parameter("x", [N, D], F32, isOutput=False)
    mask_ext = nc.declare_dram_parameter("mask", [N], F32, isOutput=False)
    wq_ext = nc.declare_dram_parameter("Wq", [H, D, E], F32, isOutput=False)
    wk_ext = nc.declare_dram_parameter("Wk", [H, D, E], F32, isOutput=False)
    wv_ext = nc.declare_dram_parameter("Wv", [H, D, E], F32, isOutput=False)
    wg_ext = nc.declare_dram_parameter("Wg", [H, D, E], F32, isOutput=False)
    ow_ext = nc.declare_dram_parameter("out_w", [E * H, E], F32, isOutput=False)
    out_ext = nc.declare_dram_parameter("out", [Nr, E], F32, isOutput=True)

    with tile.TileContext(nc) as tc, ExitStack() as ctx:
        const = ctx.enter_context(tc.tile_pool(name="const", bufs=1))
        wpool = ctx.enter_context(tc.tile_pool(name="w", bufs=1))
        hpool = ctx.enter_context(tc.tile_pool(name="head", bufs=2))
        small = ctx.enter_context(tc.tile_pool(name="small", bufs=4))
        epi = ctx.enter_context(tc.tile_pool(name="epi", bufs=3))
        outp = ctx.enter_context(tc.tile_pool(name="outp", bufs=3))
        # PSUM: wide 2x(2 banks) + po 2x(1) + tp 2x(1) = 8 banks
        psW = ctx.enter_context(tc.tile_pool(name="psW", bufs=2, space="PSUM"))
        psO = ctx.enter_context(tc.tile_pool(name="psO", bufs=2, space="PSUM"))
        psT = ctx.enter_context(tc.tile_pool(name="psT", bufs=2, space="PSUM"))


        I32 = mybir.dt.int32

        def emit_rsqrt(out_t, var_ap, width, newton=1):
            """out_t[:, 0:width] = 1/sqrt(var_ap) via bit-trick + Newton.

            Uses only mult/add ALU ops and int<->float output conversion
            (DVE pow/shift are not in the hardware ISA).
            """
            th = small.tile([128, width], I32, tag="rs_h")
            nc.vector.tensor_single_scalar(out=th, in_=var_ap.bitcast(I32),
                                           scalar=0.5, op=ALU.mult)
            y0i = small.tile([128, width], I32, tag="rs_y")
            nc.vector.tensor_scalar(out=y0i, in0=th, scalar1=-1.0,
                                    scalar2=1597463007.0, op0=ALU.mult,
                                    op1=ALU.add)
            y = y0i.bitcast(F32)
            for _ in range(newton):
                t2 = small.tile([128, width], F32, tag="rs_t2")
                nc.vector.tensor_tensor(out=t2, in0=y, in1=y, op=ALU.mult)
                nc.vector.tensor_tensor(out=t2, in0=t2, in1=var_ap,
                                        op=ALU.mult)
                nc.vector.tensor_scalar(out=t2, in0=t2, scalar1=-0.5,
                                        scalar2=1.5, op0=ALU.mult,
                                        op1=ALU.add)
                nc.vector.tensor_tensor(out=out_t[:, 0:width], in0=y,
                                        in1=t2, op=ALU.mult)
                y = out_t[:, 0:width]

        identb = const.tile([128, 128], BF16)
        make_identity(nc, identb)

        # ---- mask preprocessing ----
        mask_sb = const.tile([128, MT], F32)
        nc.sync.dma_start(out=mask_sb,
                          in_=mask_ext.ap().rearrange("(t p) -> p t", p=128))
        m01 = const.tile([128, MT], F32)
        nc.vector.tensor_single_scalar(out=m01, in_=mask_sb, scalar=0.0,
                                       op=ALU.is_ge)
        # exp bias per key tile: (m01-1)*1e9  (0 valid, -1e9 masked)
        madd = const.tile([128, MT], F32)
        nc.vector.tensor_scalar(out=madd, in0=m01, scalar1=1.0, scalar2=1e9,
                                op0=ALU.subtract, op1=ALU.mult)
        # half-mask for the query rows: 0.5*m01 (folds the tanh-gate 0.5)
        m05 = const.tile([128, MT], F32)
        nc.vector.tensor_single_scalar(out=m05, in_=m01, scalar=0.5,
                                       op=ALU.mult)

        # out_w rows permuted to k' = h*E + e ordering (row k = e*H + h).
        ow_f32 = const.tile([128, H, 2, E], F32)
        nc.gpsimd.dma_start(
            out=ow_f32,
            in_=ow_ext.ap().rearrange("(j p h) d -> p h j d", h=H, p=128))
        ow_sb = const.tile([128, H, 2, E], BF16)
        nc.gpsimd.tensor_copy(out=ow_sb, in_=ow_f32)

        for _rep in range(n_reps):
            # ---- input LN (pipelined in 4 chunks of 4 tiles) ----
            xt_all = const.tile([128, MT, D], F32)
            xbf = const.tile([128, NRT, D], BF16)
            mvx = const.tile([128, 2 * MT], F32)
            rsx = const.tile([128, MT], F32)
            nmx = const.tile([128, MT], F32)
            xnb = const.tile([128, MT, D], BF16)
            x_reshaped = x_ext.ap().rearrange("(t p) e -> p t e", p=128)
            for c4 in range(4):
                t0 = 4 * c4
                nc.sync.dma_start(out=xt_all[:, t0:t0 + 4, :],
                                  in_=x_reshaped[:, t0:t0 + 4, :])
                for t in range(t0, t0 + 4):
                    st = small.tile([128, 6], F32, tag="st")
                    nc.vector.bn_stats(out=st, in_=xt_all[:, t, :])
                    nc.vector.bn_aggr(out=mvx[:, 2 * t:2 * t + 2], in_=st)
                vv4 = small.tile([128, 4], F32, tag="vv4")
                nc.vector.tensor_copy(
                    out=vv4,
                    in_=mvx[:, 2 * t0:2 * t0 + 8]
                    .rearrange("p (t c) -> p c t", c=2)[:, 1, :])
                emit_rsqrt(rsx[:, t0:t0 + 4], vv4[:, :], 4)
                nc.vector.scalar_tensor_tensor(
                    out=nmx[:, t0:t0 + 4],
                    in0=mvx[:, 2 * t0:2 * t0 + 8]
                    .rearrange("p (t c) -> p c t", c=2)[:, 0, :],
                    scalar=-1.0, in1=rsx[:, t0:t0 + 4],
                    op0=ALU.mult, op1=ALU.mult)
                for t in range(t0, t0 + 4):
                    w = t % 4
                    if w == 0:
                        nc.scalar.activation(
                            out=xnb[:, t, :], in_=xt_all[:, t, :],
                            func=AF.Identity, scale=rsx[:, t:t + 1],
                            bias=nmx[:, t:t + 1])
                    elif w in (1, 3):
                        eng = nc.gpsimd if pool_level >= 2 else (
                            nc.vector if w == 1 else nc.scalar)
                        if eng is nc.scalar:
                            nc.scalar.activation(
                                out=xnb[:, t, :], in_=xt_all[:, t, :],
                                func=AF.Identity, scale=rsx[:, t:t + 1],
                                bias=nmx[:, t:t + 1])
                        else:
                            eng.tensor_scalar(
                                out=xnb[:, t, :], in0=xt_all[:, t, :],
                                scalar1=mvx[:, 2 * t:2 * t + 1],
                                scalar2=rsx[:, t:t + 1],
                                op0=ALU.subtract, op1=ALU.mult)
                    else:
                        nc.vector.tensor_scalar(
                            out=xnb[:, t, :], in0=xt_all[:, t, :],
                            scalar1=mvx[:, 2 * t:2 * t + 1],
                            scalar2=rsx[:, t:t + 1],
                            op0=ALU.subtract, op1=ALU.mult)
                if c4 < 2:
                    eng = nc.gpsimd if pool_level >= 1 else nc.vector
                    eng.tensor_copy(
                        out=xbf[:, t0:t0 + 4, :], in_=xt_all[:, t0:t0 + 4, :])

            xnT = const.tile([128, 2, N], FP8)
            for tp4 in range(4):   # 4 psum pack tiles, 8 transposes each
                pst = psT.tile([128, 8, 128], BF16, tag="tp")
                for i in range(4):
                    t = 4 * tp4 + i
                    for j in range(2):
                        nc.tensor.transpose(
                            pst[:, i * 2 + j, :],
                            xnb[:, t, j * 128:(j + 1) * 128], identb)
                out_ap = (xnT[:, :, :]
                          .rearrange("p j (u m) -> p u j m", m=128)
                          [:, 4 * tp4:4 * tp4 + 4, :, :])
                in_ap = pst[:, :, :].rearrange("p (u j) m -> p u j m", j=2)
                if tp4 % 2 == 0:
                    nc.vector.tensor_copy(out=out_ap, in_=in_ap)
                else:
                    nc.scalar.copy(out=out_ap, in_=in_ap)

            # ---- weights for ALL heads: staged f32 -> fp8 (Pool) ----
            w8_all = {}
            for wname, w_ext in (("q", wq_ext), ("k", wk_ext),
                                 ("v", wv_ext), ("g", wg_ext)):
                wf = wpool.tile([128, H, 2, E], F32, tag="wf_stage",
                                bufs=2)
                nc.gpsimd.dma_start(
                    out=wf,
                    in_=w_ext.ap().rearrange("h (c p) e -> p h c e", p=128))
                w8t = wpool.tile([128, H, 2, E], FP8, tag=f"w8{wname}")
                (nc.gpsimd if pool_level >= 1 else nc.vector
                 ).tensor_copy(out=w8t, in_=wf)
                w8_all[wname] = w8t

            # ---- per-head attention, software-pipelined projections ----
            resT = const.tile([128, H, 2, Nr], BF16)
            conv_i = [0]

            def conv_engine():
                conv_i[0] += 1
                return (nc.scalar if conv_i[0] % conv_act_every == 0 else nc.vector)

            def proj_thunks(h):
                """Allocate head-h tiles; return (tiles, emit-thunks)."""
                kT = hpool.tile([128, 2, N], FP8, tag="kT")
                qT = hpool.tile([128, 2, Nr], FP8, tag="qT")
                v8 = hpool.tile([128, 8, 2, E + 2], FP8, tag="v8")
                tg = hpool.tile([128, 2, 1024], BF16, tag="tg")
                thunks = []

                def ones():
                    nc.gpsimd.memset(v8[:, :, :, E:E + 2].bitcast(U8), 56)
                thunks.append(ones)

                def kt_quarter(j, mbp):
                    ps = psW.tile([128, 1024], F32, tag="wide")
                    for i in range(2):
                        mb = 2 * mbp + i
                        nc.tensor.matmul(
                            ps[:, i * 512:(i + 1) * 512],
                            w8_all["k"][:, h, :, j * 128:(j + 1) * 128],
                            xnT[:, :, mb * 512:(mb + 1) * 512],
                            start=True, stop=True, perf_mode=DR)
                    eng = conv_engine()
                    if eng is nc.vector:
                        nc.vector.tensor_copy(
                            out=kT[:, j, mbp * 1024:(mbp + 1) * 1024], in_=ps)
                    else:
                        nc.scalar.copy(
                            out=kT[:, j, mbp * 1024:(mbp + 1) * 1024], in_=ps)
                for j in range(2):
                    for mbp in range(2):
                        thunks.append(lambda j=j, mbp=mbp: kt_quarter(j, mbp))

                def qt_half(j):
                    ps = psW.tile([128, 1024], F32, tag="wide")
                    for i in range(2):
                        nc.tensor.matmul(
                            ps[:, i * 512:(i + 1) * 512],
                            w8_all["q"][:, h, :, j * 128:(j + 1) * 128],
                            xnT[:, :, i * 512:(i + 1) * 512],
                            start=True, stop=True, perf_mode=DR)
                    eng = conv_engine()
                    if eng is nc.vector:
                        nc.vector.tensor_copy(out=qT[:, j, :], in_=ps)
                    else:
                        nc.scalar.copy(out=qT[:, j, :], in_=ps)
                for j in range(2):
                    thunks.append(lambda j=j: qt_half(j))

                def v_quad(q4):
                    ps = psW.tile([128, 1024], F32, tag="wide")
                    for i in range(4):
                        t = 4 * q4 + i
                        nc.tensor.matmul(
                            ps[:, i * 256:(i + 1) * 256],
                            xnT[:, :, t * 128:(t + 1) * 128],
                            w8_all["v"][:, h],
                            start=True, stop=True, perf_mode=DR)
                    out_ap = v8[:, 2 * q4:2 * q4 + 2, :, 0:E]
                    in_ap = ps[:, :].rearrange("p (a u e) -> p a u e",
                                               a=2, u=2)
                    eng = conv_engine()
                    if eng is nc.vector:
                        nc.vector.tensor_copy(out=out_ap, in_=in_ap)
                    else:
                        nc.scalar.copy(out=out_ap, in_=in_ap)
                for q4 in range(4):
                    thunks.append(lambda q4=q4: v_quad(q4))

                def gate_half(gp):
                    ps = psW.tile([128, 1024], F32, tag="wide")
                    for i in range(4):
                        g = 4 * gp + i
                        nc.tensor.matmul(
                            ps[:, i * 256:(i + 1) * 256],
                            xnT[:, :, g * 128:(g + 1) * 128],
                            w8_all["g"][:, h],
                            start=True, stop=True, perf_mode=DR)
                    nc.scalar.activation(out=tg[:, gp, :], in_=ps,
                                         func=AF.Tanh, scale=0.5)
                for gp in range(2):
                    thunks.append(lambda gp=gp: gate_half(gp))

                return (kT, qT, v8, tg), thunks

            cur, thunks0 = proj_thunks(0)
            for th in thunks0:
                th()
            for h in range(H):
                kT, qT, v8, tg = cur
                if h + 1 < H:
                    nxt, nthunks = proj_thunks(h + 1)
                else:
                    nxt, nthunks = None, []

                # -- scores + P^T (exp), proj(h+1) interleaved --
                pts = hpool.tile([128, 8, 2, 1024], FP8, tag="pts")
                for t in range(MT):
                    ps = psW.tile([128, 1024], F32, tag="wide")
                    for i in range(2):
                        nc.tensor.matmul(
                            ps[:, i * 512:(i + 1) * 512],
                            kT[:, :, t * 128:(t + 1) * 128],
                            qT[:, :, i * 512:(i + 1) * 512],
                            start=True, stop=True, perf_mode=DR)
                    nc.scalar.activation(out=pts[:, t // 2, t % 2, :],
                                         in_=ps, func=AF.Exp,
                                         bias=madd[:, t:t + 1], scale=scale)
                    if t >= 2 and nthunks:
                        nthunks.pop(0)()
                while nthunks:
                    nthunks.pop(0)()

                # -- att@v + epilogue per 128-query group --
                a4h = hpool.tile([128, NRT, E], BF16, tag="a4h")
                mvh = small.tile([128, 2 * NRT], F32, tag="mvh")
                for g in range(NRT):
                    po = psO.tile([128, E + 2], F32, tag="po")
                    for tp in range(8):
                        nc.tensor.matmul(
                            po, pts[:, tp, :, g * 128:(g + 1) * 128],
                            v8[:, tp], start=(tp == 0), stop=(tp == 7),
                            perf_mode=DR)
                    rden = small.tile([128, 1], F32, tag="rden")
                    nc.vector.reciprocal(out=rden, in_=po[:, E:E + 1])
                    rmask = small.tile([128, 1], F32, tag="rmask")
                    nc.vector.tensor_scalar_mul(out=rmask, in0=rden,
                                                scalar1=m05[:, g:g + 1])
                    u = epi.tile([128, E], BF16, tag="u")
                    nc.vector.scalar_tensor_tensor(
                        out=u, in0=tg[:, g // 4, (g % 4) * 256:(g % 4) * 256 + 256],
                        scalar=1.0, in1=po[:, 0:E], op0=ALU.add, op1=ALU.mult)
                    nc.vector.scalar_tensor_tensor(
                        out=a4h[:, g, :], in0=u, scalar=rmask,
                        in1=xbf[:, g, :], op0=ALU.mult, op1=ALU.add)
                    st = small.tile([128, 6], F32, tag="st")
                    nc.vector.bn_stats(out=st, in_=a4h[:, g, :])
                    nc.vector.bn_aggr(out=mvh[:, 2 * g:2 * g + 2], in_=st)

                # -- head epilogue: batched rstd, applies, transposes --
                vvh = small.tile([128, NRT], F32, tag="vvh")
                nc.vector.tensor_copy(
                    out=vvh,
                    in_=mvh[:, :].rearrange("p (g c) -> p c g", c=2)[:, 1, :])
                rsh = small.tile([128, NRT], F32, tag="rsh")
                emit_rsqrt(rsh, vvh[:, :], NRT)
                res_h = hpool.tile([128, NRT, E], BF16, tag="res_h")
                nmh = small.tile([128, NRT], F32, tag="nmh")
                nc.vector.scalar_tensor_tensor(
                    out=nmh,
                    in0=mvh[:, :].rearrange("p (g c) -> p c g", c=2)[:, 0, :],
                    scalar=-1.0, in1=rsh, op0=ALU.mult, op1=ALU.mult)
                for g in range(NRT):
                    if pool_level >= 2:
                        nc.gpsimd.tensor_scalar(
                            out=res_h[:, g, :], in0=a4h[:, g, :],
                            scalar1=mvh[:, 2 * g:2 * g + 1],
                            scalar2=rsh[:, g:g + 1],
                            op0=ALU.subtract, op1=ALU.mult)
                    elif g % 2 == 0:
                        nc.scalar.activation(
                            out=res_h[:, g, :], in_=a4h[:, g, :],
                            func=AF.Identity, scale=rsh[:, g:g + 1],
                            bias=nmh[:, g:g + 1])
                    else:
                        nc.vector.tensor_scalar(
                            out=res_h[:, g, :], in0=a4h[:, g, :],
                            scalar1=mvh[:, 2 * g:2 * g + 1],
                            scalar2=rsh[:, g:g + 1],
                            op0=ALU.subtract, op1=ALU.mult)
                for g in range(NRT):
                    for j in range(2):
                        nc.sync.dma_start_transpose(
                            out=resT[:, h, j, g * 128:(g + 1) * 128],
                            in_=res_h[:, g, j * 128:(j + 1) * 128])
                cur = nxt

            # ---- output projection + final LN (per-group, 2 psum bufs) ----
            for g in range(NRT):
                ps = psO.tile([128, E + 2], F32, tag="po")
                for h in range(H):
                    for j in range(2):
                        nc.tensor.matmul(
                            ps[:, 0:E],
                            resT[:, h, j, g * 128:(g + 1) * 128],
                            ow_sb[:, h, j, :],
                            start=(h == 0 and j == 0), stop=False)
                # residual: += I @ x_bf16
                nc.tensor.matmul(ps[:, 0:E], identb, xbf[:, g, :],
                                 start=False, stop=True)
                st = small.tile([128, 6], F32, tag="st")
                nc.vector.bn_stats(out=st, in_=ps[:, 0:E])
                mvo = small.tile([128, 2], F32, tag="mvo")
                nc.vector.bn_aggr(out=mvo, in_=st)
                rso = small.tile([128, 1], F32, tag="rso")
                emit_rsqrt(rso, mvo[:, 1:2], 1)
                # scale = rstd*m01[g], bias = -mean*scale
                rmo = small.tile([128, 1], F32, tag="rmo")
                nc.vector.tensor_scalar_mul(out=rmo, in0=rso[:, 0:1],
                                            scalar1=m01[:, g:g + 1])
                nmo = small.tile([128, 1], F32, tag="nmo")
                nc.vector.scalar_tensor_tensor(
                    out=nmo, in0=mvo[:, 0:1], scalar=-1.0, in1=rmo,
                    op0=ALU.mult, op1=ALU.mult)
                o_t = outp.tile([128, E], F32, tag="o_t")
                nc.scalar.activation(out=o_t, in_=ps[:, 0:E],
                                     func=AF.Identity,
                                     scale=rmo, bias=nmo)
                # (kept on Act: reads PSUM; DVE is stats-bound here)
                nc.sync.dma_start(out=out_ext.ap()[g * 128:(g + 1) * 128, :],
                                  in_=o_t)

    nc.compile()
    return nc


def _get_nc(n_reps=1):
    key = ("nc", n_reps, tuple(sorted(CFG.items())))
    if key not in _cache:
        _cache[key] = _build(n_reps)
    return _cache[key]


def _make_in_maps(inputs):
    f = lambda a: np.ascontiguousarray(np.asarray(a), dtype=np.float32)
    x, mask = f(inputs["x"]), f(inputs["mask"])
    Wq, Wk = f(inputs["Wq"]), f(inputs["Wk"])
    Wv, Wg = f(inputs["Wv"]), f(inputs["Wg"])
    out_w = f(inputs["out_w"])
    in_maps = []
    for c in range(NCORES):
        b, r = c // 2, c % 2
        in_maps.append({
            "x": np.roll(x[b], -r * NR, axis=0) if r else x[b],
            "mask": np.roll(mask[b], -r * NR) if r else mask[b],
            "Wq": Wq, "Wk": Wk, "Wv": Wv, "Wg": Wg, "out_w": out_w,
        })
    return in_maps


def _get_exec(n_reps=1):
    """Compile (once) and cache a jitted 8-core executor for the SPMD graph.

    Returns (fn, pack) where fn(*concat_arrays) -> tuple of concat outputs
    and pack describes the parameter order.
    """
    key = ("exec", n_reps, tuple(sorted(CFG.items())))
    if key in _cache:
        return _cache[key]

    import jax
    from jax.sharding import Mesh, PartitionSpec
    from jax.experimental.shard_map import shard_map
    import concourse.mybir as mybir
    from concourse import bass2jax
    from concourse.bass2jax import _bass_exec_p, install_neuronx_cc_hook

    nc = _get_nc(n_reps)
    install_neuronx_cc_hook()

    partition_name = (nc.partition_id_tensor.name
                      if nc.partition_id_tensor else None)
    in_names, out_names, out_avals, zero_outs = [], [], [], []
    for alloc in nc.m.functions[0].allocations:
        if not isinstance(alloc, mybir.MemoryLocationSet):
            continue
        name = alloc.memorylocations[0].name
        if alloc.kind == "ExternalInput":
            if name != partition_name:
                in_names.append(name)
        elif alloc.kind == "ExternalOutput":
            out_names.append(name)
            shape = tuple(alloc.tensor_shape)
            dtype = mybir.dt.np(alloc.dtype)
            out_avals.append(jax.core.ShapedArray(shape, dtype))
            zero_outs.append(np.zeros(shape, dtype))
    n_params = len(in_names)
    all_in_names = list(in_names) + list(out_names)
    if partition_name is not None:
        all_in_names.append(partition_name)

    def _body(*args):
        operands = list(args)
        if partition_name is not None:
            operands.append(bass2jax.partition_id_tensor())
        outs = _bass_exec_p.bind(
            *operands,
            out_avals=tuple(out_avals),
            in_names=tuple(all_in_names),
            out_names=tuple(out_names),
            lowering_input_output_aliases=(),
            sim_require_finite=True,
            sim_require_nnan=True,
            nc=nc,
        )
        return tuple(outs)

    try:
        devices = jax.devices("axon")[:NCORES]
    except RuntimeError:
        devices = jax.devices()[:NCORES]
    assert len(devices) >= NCORES, f"need {NCORES} devices, have {len(devices)}"
    mesh = Mesh(np.asarray(devices), ("core",))
    in_specs = (PartitionSpec("core"),) * (n_params + len(out_names))
    out_specs = (PartitionSpec("core"),) * len(out_names)
    fn = jax.jit(shard_map(_body, mesh=mesh, in_specs=in_specs,
                           out_specs=out_specs, check_rep=False),
                 donate_argnums=tuple(range(n_params,
                                            n_params + len(out_names))),
                 keep_unused=True)
    pack = (in_names, out_names, out_avals, zero_outs, n_params)
    _cache[key] = (fn, pack)
    return fn, pack


def _concat_args(in_maps, pack):
    in_names, out_names, out_avals, zero_outs, n_params = pack
    per_core = [[np.asarray(m[name]) for name in in_names] for m in in_maps]
    concat_in = [np.concatenate([per_core[c][i] for c in range(NCORES)], axis=0)
                 for i in range(n_params)]
    concat_zeros = [np.zeros((NCORES * z.shape[0], *z.shape[1:]), z.dtype)
                    for z in zero_outs]
    return concat_in + concat_zeros


def _run_exec(inputs, n_reps=1):
    import jax
    fn, pack = _get_exec(n_reps)
    in_maps = _make_in_maps(inputs)
    args = _concat_args(in_maps, pack)
    out_arrs = fn(*args)
    jax.block_until_ready(out_arrs)
    out_avals = pack[2]
    o = np.asarray(out_arrs[0]).reshape(NCORES, *out_avals[0].shape)
    out = np.empty((B, N, E), np.float32)
    for c in range(NCORES):
        b, r = c // 2, c % 2
        out[b, r * NR:(r + 1) * NR] = o[c]
    return out


def kernel(**inputs):
    """Full-input entry point: shard, run SPMD on cores 0-7, gather."""
    from concourse.bass_utils import run_bass_kernel_spmd
    nc = _get_nc(1)
    in_maps = _make_in_maps(inputs)
    res = run_bass_kernel_spmd(nc, in_maps, core_ids=list(range(NCORES)))
    out = np.empty((B, N, E), np.float32)
    for c in range(NCORES):
        b, r = c // 2, c % 2
        out[b, r * NR:(r + 1) * NR] = res.results[c]["out"]
    return out
